# revision 38
# baseline (speedup 1.0000x reference)
"""Trainium2 Bass kernel for nn_Encoder_30897994727668.

Reference computes (no recurrence, so every timestep independent):
    gates = x @ W_ih.T + b_ih + b_hh            # [B,T,4H], gate order i,f,g,o
    c = sigmoid(i) * tanh(g)                    # f gate unused (c_prev = 0)
    h = sigmoid(o) * tanh(c)
    return (h, c)

Kernel strategy (pure data parallel over B*T across 8 cores):
  * Skip the f gate entirely (never used) -> 768 of 1024 gate rows.
  * Fold sigmoid into tanh: sigmoid(z) = (1 + tanh(z/2))/2, by pre-scaling
    the i/o rows of W and b by 0.5 on the host. Then ONE ScalarE tanh pass
    covers all three gates of a tile.
  * fp16 matmul operands (fp32 matmul runs as 2 hi/lo passes on the PE --
    2x cost; fp16 streams 1 col/cycle like bf16 but with 11-bit mantissa),
    fp32 PSUM accumulation.
  * gates layout [token(partition=128), gate(free=768)]: per 128-token tile,
    PE-transpose x tile (tokens x feat -> feat x tokens), then
    matmul(lhsT=xT, rhs=W'^T). Bias is accumulated into PSUM with a second
    matmul (lhsT = ones[128,128], rhs has bias in row 0) -- avoids any
    fp32 tensor_tensor bias pass.
  * Two token-tiles share one 3-bank PSUM tile [128,2,768] so the ScalarE
    tanh reads FD=1536 per op (amortizes the ~172cyc/op PSUM overhead).
    The shared middle bank needs one manual ordering dep per pair.
  * Post-activation math in fp16 on VectorE (2x/4x perf modes):
       c = (0.5*ti + 0.5) * tg ;  u = tanh(c) on ScalarE ;  h = (0.5*to+0.5)*u
  * h and c stay fp16 in SBUF; the output DMA (SWDGE) casts to fp32 on the
    way to DRAM.
  * Token <-> partition mapping t = macro*2048 + p*16 + k keeps both the
    x loads and the h/c stores fully contiguous per partition.
"""

import sys

if "/opt/trn_rl_repo" not in sys.path:
    sys.path.insert(0, "/opt/trn_rl_repo")

import numpy as np

import concourse.bacc as bacc
import concourse.bass as bass
import concourse.tile as tile
from concourse import mybir
from concourse.bass_utils import run_bass_kernel_spmd
from concourse.masks import make_identity
from concourse.tile_rust import add_dep_helper

N_CORES = 8
BATCH = 64
SEQ = 2048
IN = 128          # input features (= K of the matmul = partition count)
H = 256           # hidden
G = 3 * H         # gates kept: i, g, o  (f skipped)
TOKENS = BATCH * SEQ              # 131072
TOK_PER_CORE = TOKENS // N_CORES  # 16384
MACRO_TOK = 2048                  # tokens per macro-iteration
TILES = MACRO_TOK // 128          # 16 token-tiles per macro
PAIRS = TILES // 2
MACROS = TOK_PER_CORE // MACRO_TOK  # 8

F32 = mybir.dt.float32
F16 = mybir.dt.float16
BF16 = mybir.dt.bfloat16


def _build_program():
    nc = bacc.Bacc(None, target_bir_lowering=False, debug=False)

    x_d = nc.dram_tensor("x", [TOK_PER_CORE, IN], F32, kind="ExternalInput")
    wt_d = nc.dram_tensor("wt", [IN, G], F16, kind="ExternalInput")
    bias_d = nc.dram_tensor("bias", [G], F16, kind="ExternalInput")
    h_d = nc.dram_tensor("h", [TOK_PER_CORE, H], F32, kind="ExternalOutput")
    c_d = nc.dram_tensor("c", [TOK_PER_CORE, H], F32, kind="ExternalOutput")

    AF = mybir.ActivationFunctionType
    OP = mybir.AluOpType

    with tile.TileContext(nc) as tc:
        with (
            tc.tile_pool(name="consts", bufs=1) as consts,
            tc.tile_pool(name="xin", bufs=3) as xin,
            tc.tile_pool(name="xt", bufs=3) as xtp,
            tc.tile_pool(name="tst", bufs=2) as tstp,
            tc.tile_pool(name="wv", bufs=2) as wvp,
            tc.tile_pool(name="outs", bufs=2) as outp,
            tc.tile_pool(name="ps_t", bufs=2, space=bass.MemorySpace.PSUM) as ps_t,
            tc.tile_pool(name="ps_g", bufs=2, space=bass.MemorySpace.PSUM) as ps_g,
        ):
            # ---- constants (DMAs for wt/bias issued after macro 0's x) ----
            ident = consts.tile([128, 128], F32)
            make_identity(nc, ident)
            ones = consts.tile([128, 128], F16)
            nc.vector.memset(ones, 1.0)
            wt_sb = consts.tile([IN, G], F16)
            nc.sync.dma_start(wt_sb[:], wt_d[:])
            bias_b = consts.tile([128, G], F16)
            nc.vector.memset(bias_b, 0.0)
            nc.sync.dma_start(
                bias_b[0:1, :], bass.AP(bias_d, 0, [[0, 1], [1, G]])
            )

            # ---------------------------------------------------------------
            # Global chunk loop, software-pipelined so the ScalarE queue
            # never stalls: chunk q's gate-matmuls+tanh are issued first,
            # postlude stage A (w, c, tanh(c)) one chunk behind, stage B
            # (v, h) two chunks behind.
            # ---------------------------------------------------------------
            CHUNK_TILES = 4
            CHUNKS_PER_MACRO = TILES // CHUNK_TILES      # 4
            NCHUNKS = MACROS * CHUNKS_PER_MACRO          # 32

            x_tiles = [None] * MACROS
            t_tiles = [None] * MACROS
            c_tiles = [None] * MACROS
            h_tiles = [None] * MACROS
            u_tiles = [None] * NCHUNKS

            def load_macro(mac):
                t0 = mac * MACRO_TOK
                # x_st[p, k, i] = x[t0 + p*TILES + k, i]; two half-loads so
                # the first chunk's compute can start sooner
                x_st = xin.tile([128, TILES, IN], F32, tag="x", name=f"xst{mac}")
                x_view = x_d[t0 : t0 + MACRO_TOK, :].rearrange(
                    "(p k) i -> p k i", k=TILES
                )
                # macro 0 loads in quarters so the very first chunk's
                # compute starts as early as possible
                step = TILES // (4 if mac == 0 else 2)
                for lo in range(0, TILES, step):
                    nc.sync.dma_start(
                        x_st[:, lo : lo + step, :], x_view[:, lo : lo + step, :]
                    )
                x_tiles[mac] = x_st
                t_tiles[mac] = tstp.tile([128, TILES, G], F16, tag="t", name=f"tst{mac}")
                c_tiles[mac] = outp.tile([128, TILES, H], F16, tag="c", name=f"cst{mac}")
                h_tiles[mac] = outp.tile([128, TILES, H], F16, tag="h", name=f"hst{mac}")

            def emit_pair(mac, kp):
                x_st, t_st = x_tiles[mac], t_tiles[mac]
                g_ps = ps_g.tile([128, 2, G], F32)  # 3 PSUM banks
                mid_bank_clearer = None
                for j in (0, 1):
                    k = 2 * kp + j
                    xT_ps = ps_t.tile([128, 128], F32)
                    nc.tensor.transpose(xT_ps[:], x_st[:, k, :], ident[:])
                    xT_sb = xtp.tile([128, 128], F16)
                    nc.vector.tensor_copy(xT_sb[:], xT_ps[:])

                    # bank-aligned matmul split: tile0 -> 512|256,
                    # tile1 -> 256|512 (pair spans banks b|b+1|b+2)
                    cuts = [(0, 512, True), (512, 768, True)] if j == 0 else [
                        (0, 256, False), (256, 768, True)]
                    for lo, hi, starts in cuts:
                        mm = nc.tensor.matmul(
                            g_ps[:, j, lo:hi], xT_sb[:], wt_sb[:, lo:hi],
                            start=starts, stop=False, skip_group_check=True,
                        )
                        if j == 0 and lo == 512:
                            # clears has_written for the shared middle bank;
                            # tile1's first mm must come after
                            mid_bank_clearer = mm
                        if j == 1 and lo == 0:
                            add_dep_helper(
                                mm.ins,
                                mid_bank_clearer.ins,
                                reason="shared PSUM bank: overwrite after clear",
                            )
                        nc.tensor.matmul(
                            g_ps[:, j, lo:hi], ones[:], bias_b[:, lo:hi],
                            start=False, stop=True, skip_group_check=True,
                        )
                # one tanh pass over both tiles' [i'|g|o'] (FD=1536)
                nc.scalar.activation(
                    t_st[:, 2 * kp : 2 * kp + 2, :], g_ps[:], AF.Tanh
                )

            def emit_post_a(q):
                mac, ci = q // CHUNKS_PER_MACRO, q % CHUNKS_PER_MACRO
                sl = slice(ci * CHUNK_TILES, (ci + 1) * CHUNK_TILES)
                t_st = t_tiles[mac]
                ti = t_st[:, sl, 0:H]
                tg = t_st[:, sl, H : 2 * H]
                w = wvp.tile([128, CHUNK_TILES, H], F16, tag="wv")
                nc.vector.tensor_scalar(w[:], ti, 0.5, 0.5, OP.mult, OP.add)
                c_sl = c_tiles[mac][:, sl, :]
                nc.vector.tensor_mul(c_sl, w[:], tg)
                u = wvp.tile([128, CHUNK_TILES, H], F16, tag="u", name=f"u{q}")
                nc.scalar.activation(u[:], c_sl, AF.Tanh)
                u_tiles[q] = u

            def emit_post_b(q):
                mac, ci = q // CHUNKS_PER_MACRO, q % CHUNKS_PER_MACRO
                sl = slice(ci * CHUNK_TILES, (ci + 1) * CHUNK_TILES)
                to = t_tiles[mac][:, sl, 2 * H : 3 * H]
                v = wvp.tile([128, CHUNK_TILES, H], F16, tag="wv")
                nc.vector.tensor_scalar(v[:], to, 0.5, 0.5, OP.mult, OP.add)
                nc.vector.tensor_mul(h_tiles[mac][:, sl, :], v[:], u_tiles[q][:])
                # store half-macros as soon as their tiles are done; the last
                # macro stores per-chunk to shorten the kernel tail
                per_chunk = mac == MACROS - 1
                if per_chunk or ci % 2 == 1:
                    t0 = mac * MACRO_TOK
                    if per_chunk:
                        hsl = sl
                    else:
                        half = TILES // 2
                        hsl = slice(0, half) if ci == 1 else slice(half, TILES)
                    h_view = h_d[t0 : t0 + MACRO_TOK, :].rearrange(
                        "(p k) j -> p k j", k=TILES
                    )
                    c_view = c_d[t0 : t0 + MACRO_TOK, :].rearrange(
                        "(p k) j -> p k j", k=TILES
                    )
                    nc.gpsimd.dma_start(h_view[:, hsl, :], h_tiles[mac][:, hsl, :])
                    nc.gpsimd.dma_start(c_view[:, hsl, :], c_tiles[mac][:, hsl, :])

            load_macro(0)
            for q in range(NCHUNKS + 2):
                if q < NCHUNKS:
                    mac, ci = q // CHUNKS_PER_MACRO, q % CHUNKS_PER_MACRO
                    # prefetch next macro's x mid-way through this one
                    if ci == 0 and mac + 1 < MACROS:
                        load_macro(mac + 1)
                    for kp in range(
                        ci * CHUNK_TILES // 2, (ci + 1) * CHUNK_TILES // 2
                    ):
                        emit_pair(mac, kp)
                if 1 <= q and q - 1 < NCHUNKS:
                    emit_post_a(q - 1)
                if 2 <= q and q - 2 < NCHUNKS:
                    emit_post_b(q - 2)

    nc.compile()
    return nc


_NC_CACHE = None


def _get_nc():
    global _NC_CACHE
    if _NC_CACHE is None:
        _NC_CACHE = _build_program()
    return _NC_CACHE


def _prep_weights(W_ih, b_ih, b_hh):
    W = np.asarray(W_ih, dtype=np.float32)
    b = np.asarray(b_ih, dtype=np.float32) + np.asarray(b_hh, dtype=np.float32)
    Wi, Wg, Wo = W[0:H], W[2 * H : 3 * H], W[3 * H : 4 * H]
    bi, bg, bo = b[0:H], b[2 * H : 3 * H], b[3 * H : 4 * H]
    Wp = np.concatenate([0.5 * Wi, Wg, 0.5 * Wo], axis=0)       # [768, 128]
    bp = np.concatenate([0.5 * bi, bg, 0.5 * bo], axis=0)       # [768]
    wt = np.ascontiguousarray(Wp.T).astype(np.float16)  # [128, 768]
    return wt, np.ascontiguousarray(bp).astype(np.float16)


def kernel(x, W_ih, W_hh, b_ih, b_hh):
    nc = _get_nc()
    x = np.asarray(x, dtype=np.float32).reshape(TOKENS, IN)
    wt, bp = _prep_weights(W_ih, b_ih, b_hh)

    in_maps = []
    for core in range(N_CORES):
        sl = x[core * TOK_PER_CORE : (core + 1) * TOK_PER_CORE]
        in_maps.append({"x": np.ascontiguousarray(sl), "wt": wt, "bias": bp})

    res = run_bass_kernel_spmd(nc, in_maps, core_ids=list(range(N_CORES)))

    h = np.concatenate([res.results[i]["h"] for i in range(N_CORES)], axis=0)
    c = np.concatenate([res.results[i]["c"] for i in range(N_CORES)], axis=0)
    h = h.reshape(BATCH, SEQ, H)
    c = c.reshape(BATCH, SEQ, H)
    return (h, c)
